# revision 10
# baseline (speedup 1.0000x reference)
"""Bahdanau additive attention (ragged sequence) on 8 Trainium2 NeuronCores.

Reference math (per batch b over sequence l, d=512, a=64):
    parts  = enc @ W_enc + b_attn                        (l, a)
    scores = tanh(parts + hidden @ W_hidden) . v         (l,)
    w      = softmax(scores + mask) over l               (valid: l < lens[b])
    out[b] = sum_l w[l] * enc[l, b, :]                   (512,)

Strategy (batch-parallel over 8 cores, 8 batches each; single pass over enc):
  * Host pre-transposes enc to (b, d, l) and pre-rounds it to fp32r (fp32
    with 11-bit mantissa, RNE — the PE's full-rate 4-byte mode), so stage A
    (the d-contraction) streams natural [128, 512] tiles through the PE at
    1 col/cycle with W_enc chunks stationary.
  * tanh runs on ACT (bias = b_attn + hidden@W_hidden per-partition), fp16
    out; the v-dot runs on PE with a column-replicated v [64, 128] so scores
    land broadcast across all 128 PSUM partitions. A second matmul
    accumulates a host-built -30000/0 length mask into the same PSUM slice.
  * exp runs on ACT straight out of PSUM with accum_out giving the softmax
    denominator for free; max-subtraction is unnecessary because
    |scores| <= sum|v| < 60 (exp stays finite in fp32) and the +1.0 valid
    shift in the reference cancels in softmax.
  * Stage B (the l-contraction) runs on DVE as one scalar_tensor_tensor per
    (b, d-chunk): accum_out[d] = sum_l enc[d, l] * (1/S) * P[l], which both
    applies the softmax normalization and avoids any on-device transpose.
  * The per-core result [128, 4*8] is PE-transposed once and DMA'd out.

Sync-wait constraint: matmuls with inline weight load (S3_LW struct) can
carry at most ONE semaphore wait, so every PE-consumed constant is staged
through a DVE copy (PE then only ever waits the Vector semaphore for all of
them), and enc is DMA'd in 256 KiB single-queue chunks so each stage-A
matmul waits on exactly its own chunk's DMA semaphore.
"""
import sys

sys.path.insert(0, "/opt/trn_rl_repo")

from contextlib import ExitStack

import numpy as np

import concourse.bass as bass
import concourse.bacc as bacc
import concourse.tile as tile
from concourse import mybir
from concourse.bass_utils import run_bass_kernel_spmd

F32 = mybir.dt.float32
F32R = mybir.dt.float32r
F16 = mybir.dt.float16
F8 = mybir.dt.float8e5

N_CORES = 8
L, B, D, A, H = 2048, 64, 512, 64, 512
BL = B // N_CORES  # local batches per core
DC = D // 128  # d-chunks of 128 partitions
LC = L // 512  # l-chunks of 512 (matmul moving-operand max for 4-byte)
MASK_NEG = np.float16(-30000.0)


def _round_fp32r(a: np.ndarray) -> np.ndarray:
    """Round fp32 mantissa to 11 bits (RNE) — the PE's fp32r operand format."""
    u = np.ascontiguousarray(a, dtype=np.float32).view(np.uint32)
    lsb = (u >> np.uint32(12)) & np.uint32(1)
    u = (u + np.uint32(0x7FF) + lsb) & np.uint32(0xFFFFF000)
    return u.view(np.float32)


def _build_bass():
    nc = bacc.Bacc(
        "TRN2", target_bir_lowering=False, debug=False, num_devices=N_CORES
    )
    encT = nc.dram_tensor("encT", [BL * D, L], F32R, kind="ExternalInput")
    msk = nc.dram_tensor("msk", [1, BL * L], F8, kind="ExternalInput")
    hidT = nc.dram_tensor("hidT", [H, BL], F32, kind="ExternalInput")
    w_enc = nc.dram_tensor("w_enc", [D, A], F32R, kind="ExternalInput")
    w_hid = nc.dram_tensor("w_hid", [H, A], F32, kind="ExternalInput")
    b_attn = nc.dram_tensor("b_attn", [A, 1], F32, kind="ExternalInput")
    vrep = nc.dram_tensor("vrep", [A, 128], F16, kind="ExternalInput")
    ones1 = nc.dram_tensor("ones1", [1, 128], F8, kind="ExternalInput")
    ident = nc.dram_tensor("ident", [128, 128], F32, kind="ExternalInput")
    out = nc.dram_tensor("out", [BL, D], F32, kind="ExternalOutput")

    with tile.TileContext(nc) as tc, ExitStack() as ctx:
        const = ctx.enter_context(tc.tile_pool(name="const", bufs=1))
        encp = ctx.enter_context(tc.tile_pool(name="encp", bufs=4))
        tanhp = ctx.enter_context(tc.tile_pool(name="tanhp", bufs=3))
        pp = ctx.enter_context(tc.tile_pool(name="pp", bufs=2))
        scrp = ctx.enter_context(tc.tile_pool(name="scrp", bufs=2))
        smallp = ctx.enter_context(tc.tile_pool(name="smallp", bufs=4))
        resp = ctx.enter_context(tc.tile_pool(name="resp", bufs=1))
        ps_parts = ctx.enter_context(
            tc.tile_pool(name="ps_parts", bufs=2, space="PSUM")
        )
        ps_sc = ctx.enter_context(tc.tile_pool(name="ps_sc", bufs=2, space="PSUM"))
        ps_misc = ctx.enter_context(tc.tile_pool(name="ps_misc", bufs=1, space="PSUM"))

        # ---- one-time constants (Bacc's event-sem pass handles multi-waits) ----
        def loaded(shape, dtype, dram_ap):
            dst = const.tile(shape, dtype, tag="c_" + dram_ap.tensor.name)
            nc.sync.dma_start(dst[:], dram_ap)
            return dst

        w_enc_sb = loaded(
            [128, DC, A], F32R, w_enc.ap().rearrange("(dc p) a -> p dc a", p=128)
        )
        w_hid_sb = loaded(
            [128, DC, A], F32, w_hid.ap().rearrange("(dc p) a -> p dc a", p=128)
        )
        hidT_sb = loaded(
            [128, DC, BL], F32, hidT.ap().rearrange("(dc p) b -> p dc b", p=128)
        )
        vrep_sb = loaded([A, 128], F16, vrep.ap())
        ones1_sb = loaded([1, 128], F8, ones1.ap())
        msk_sb = loaded([1, BL * L], F8, msk.ap())
        ident_sb = loaded([128, 128], F32, ident.ap())
        b_attn_sb = loaded([A, 1], F32, b_attn.ap())

        # hid = hidden @ W_hidden, transposed to [a, b] (tiny, full fp32)
        hid_ps = ps_misc.tile([A, BL], F32, tag="hid")
        for dc in range(DC):
            nc.tensor.matmul(
                hid_ps[:], lhsT=w_hid_sb[:, dc, :], rhs=hidT_sb[:, dc, :],
                start=(dc == 0), stop=(dc == DC - 1),
            )
        hplus_sb = const.tile([A, BL], F32)  # b_attn + hid.T, per-partition bias
        nc.vector.tensor_scalar_add(hplus_sb[:], hid_ps[:], b_attn_sb[:])

        res_all = resp.tile([128, BL * DC], F32)  # col j = b*DC + dc

        encT_v = encT.ap().rearrange("(b dc p) l -> b dc p l", dc=DC, p=128)

        for b in range(BL):
            et = encp.tile([128, DC, L], F32R, tag="et")
            for dc in range(DC):
                nc.sync.dma_start(et[:, dc, :], encT_v[b, dc, :, :])

            p_sb = pp.tile([128, L], F32, tag="p")
            s_half = [None, None]
            for h in range(2):
                sc_ps = ps_sc.tile([128, L // 2], F32, tag="sc")
                for half_lc in range(2):
                    lc = 2 * h + half_lc
                    lsl = slice(lc * 512, (lc + 1) * 512)
                    hsl = slice(half_lc * 512, (half_lc + 1) * 512)
                    parts_ps = ps_parts.tile([A, 512], F32, tag="parts")
                    for dc in range(DC):
                        nc.tensor.matmul(
                            parts_ps[:], lhsT=w_enc_sb[:, dc, :], rhs=et[:, dc, lsl],
                            start=(dc == 0), stop=(dc == DC - 1),
                        )
                    th = tanhp.tile([A, 512], F16, tag="th")
                    nc.scalar.activation(
                        th[:], parts_ps[:], mybir.ActivationFunctionType.Tanh,
                        bias=hplus_sb[:, b : b + 1],
                    )
                    # scores (broadcast to 128 partitions) + length mask
                    nc.tensor.matmul(
                        sc_ps[:, hsl], lhsT=vrep_sb[:], rhs=th[:],
                        start=True, stop=False,
                    )
                    nc.tensor.matmul(
                        sc_ps[:, hsl], lhsT=ones1_sb[:],
                        rhs=msk_sb[:, b * L + lc * 512 : b * L + (lc + 1) * 512],
                        start=False, stop=True,
                    )
                sh = smallp.tile([128, 1], F32, tag=f"sh{h}")
                nc.scalar.activation(
                    p_sb[:, h * 1024 : (h + 1) * 1024], sc_ps[:],
                    mybir.ActivationFunctionType.Exp, accum_out=sh[:],
                )
                s_half[h] = sh

            s_col = smallp.tile([128, 1], F32, tag="s")
            nc.vector.tensor_add(s_col[:], s_half[0][:], s_half[1][:])
            r_col = smallp.tile([128, 1], F32, tag="r")
            nc.vector.reciprocal(r_col[:], s_col[:])

            for dc in range(DC):
                scratch = scrp.tile([128, L], F32, tag="scratch")
                nc.vector.scalar_tensor_tensor(
                    out=scratch[:],
                    in0=et[:, dc, :].bitcast(F32),
                    scalar=r_col[:],
                    in1=p_sb[:],
                    op0=mybir.AluOpType.mult,
                    op1=mybir.AluOpType.mult,
                    accum_out=res_all[:, b * DC + dc : b * DC + dc + 1],
                )

        # transpose res_all -> [BL*DC, 128] and write out
        t_ps = ps_misc.tile([BL * DC, 128], F32, tag="tps")
        nc.tensor.transpose(t_ps[:], res_all[:], ident_sb[:])
        out_sb = resp.tile([BL * DC, 128], F32)
        nc.vector.tensor_copy(out_sb[:], t_ps[:])
        nc.sync.dma_start(out.ap().rearrange("b (dc x) -> (b dc) x", x=128), out_sb[:])

    nc.compile()
    return nc


_NC_CACHE = None


def _get_nc():
    global _NC_CACHE
    if _NC_CACHE is None:
        _NC_CACHE = _build_bass()
    return _NC_CACHE


def prepare_in_maps(enc_outputs, lens, hidden_states, W_enc, b_attn, W_hidden, v):
    """Host-side sharding + layout transforms. Returns list of per-core maps."""
    enc_outputs = np.asarray(enc_outputs, dtype=np.float32)
    lens = np.asarray(lens, dtype=np.int32)
    hidden_states = np.asarray(hidden_states, dtype=np.float32)
    W_enc = np.asarray(W_enc, dtype=np.float32)
    b_attn = np.asarray(b_attn, dtype=np.float32)
    W_hidden = np.asarray(W_hidden, dtype=np.float32)
    v = np.asarray(v, dtype=np.float32)

    # (L, B, D) -> (B, D, L), contiguous, pre-rounded to fp32r
    encT = _round_fp32r(np.ascontiguousarray(enc_outputs.transpose(1, 2, 0)))
    w_enc_r = _round_fp32r(W_enc)
    import ml_dtypes
    vrep = np.ascontiguousarray(np.repeat(v.astype(np.float16)[:, None], 128, axis=1))
    ones1 = np.ones((1, 128), dtype=ml_dtypes.float8_e5m2)
    ident = np.eye(128, dtype=np.float32)
    b_attn_c = np.ascontiguousarray(b_attn[:, None])

    # length mask rows: 0 where l < lens[b], -30000 where l >= lens[b]
    li = np.arange(L, dtype=np.int32)[None, :]
    mask_full = np.where(li < lens[:, None], 0.0, -30000.0).astype(
        ml_dtypes.float8_e5m2
    )  # (B, L)

    hiddenT = np.ascontiguousarray(hidden_states.T)  # (H, B)

    in_maps = []
    for c in range(N_CORES):
        bs = slice(c * BL, (c + 1) * BL)
        in_maps.append(
            {
                "encT": encT[bs].reshape(BL * D, L),
                "msk": np.ascontiguousarray(mask_full[bs].reshape(1, BL * L)),
                "hidT": np.ascontiguousarray(hiddenT[:, bs]),
                "w_enc": w_enc_r,
                "w_hid": W_hidden,
                "b_attn": b_attn_c,
                "vrep": vrep,
                "ones1": ones1,
                "ident": ident,
            }
        )
    return in_maps


def kernel(enc_outputs, lens, hidden_states, W_enc, b_attn, W_hidden, v, **kwargs):
    nc = _get_nc()
    in_maps = prepare_in_maps(
        enc_outputs, lens, hidden_states, W_enc, b_attn, W_hidden, v
    )
    res = run_bass_kernel_spmd(nc, in_maps, core_ids=list(range(N_CORES)))
    return np.concatenate([res.results[c]["out"] for c in range(N_CORES)], axis=0)


def kernel_traced(enc_outputs, lens, hidden_states, W_enc, b_attn, W_hidden, v):
    """Like kernel() but returns (output, BassKernelResults with trace)."""
    nc = _get_nc()
    in_maps = prepare_in_maps(
        enc_outputs, lens, hidden_states, W_enc, b_attn, W_hidden, v
    )
    res = run_bass_kernel_spmd(nc, in_maps, core_ids=list(range(N_CORES)), trace=True)
    out = np.concatenate([res.results[c]["out"] for c in range(N_CORES)], axis=0)
    return out, res


# revision 13
# speedup vs baseline: 1.0341x; 1.0341x over previous
"""Bahdanau additive attention (ragged sequence) on 8 Trainium2 NeuronCores.

Reference math (per batch b over sequence l, d=512, a=64):
    parts  = enc @ W_enc + b_attn                        (l, a)
    scores = tanh(parts + hidden @ W_hidden) . v         (l,)
    w      = softmax(scores + mask) over l               (valid: l < lens[b])
    out[b] = sum_l w[l] * enc[l, b, :]                   (512,)

Strategy (batch-parallel over 8 cores, 8 batches each; single pass over enc):
  * Host pre-transposes enc to (b, d, l) and pre-rounds it to fp32r (fp32
    with 11-bit mantissa, RNE — the PE's full-rate 4-byte mode), so stage A
    (the d-contraction) streams natural [128, 512] tiles through the PE at
    1 col/cycle with W_enc chunks stationary.
  * tanh runs on ACT (bias = b_attn + hidden@W_hidden per-partition), fp16
    out; the v-dot runs on PE with a column-replicated v [64, 128] so scores
    land broadcast across all 128 PSUM partitions. A second matmul
    accumulates a host-built -30000/0 length mask into the same PSUM slice.
  * exp runs on ACT straight out of PSUM with accum_out giving the softmax
    denominator for free; max-subtraction is unnecessary because
    |scores| <= sum|v| < 60 (exp stays finite in fp32) and the +1.0 valid
    shift in the reference cancels in softmax.
  * Stage B (the l-contraction) runs on DVE as one scalar_tensor_tensor per
    (b, d-chunk): accum_out[d] = sum_l enc[d, l] * (1/S) * P[l], which both
    applies the softmax normalization and avoids any on-device transpose.
  * The per-core result [128, 4*8] is PE-transposed once and DMA'd out.

Sync-wait constraint: matmuls with inline weight load (S3_LW struct) can
carry at most ONE semaphore wait, so every PE-consumed constant is staged
through a DVE copy (PE then only ever waits the Vector semaphore for all of
them), and enc is DMA'd in 256 KiB single-queue chunks so each stage-A
matmul waits on exactly its own chunk's DMA semaphore.
"""
import sys

sys.path.insert(0, "/opt/trn_rl_repo")

from contextlib import ExitStack

import numpy as np

import concourse.bass as bass
import concourse.bacc as bacc
import concourse.tile as tile
from concourse import mybir
from concourse.bass_utils import run_bass_kernel_spmd

F32 = mybir.dt.float32
F32R = mybir.dt.float32r
F16 = mybir.dt.float16
F8 = mybir.dt.float8e5

N_CORES = 8
L, B, D, A, H = 2048, 64, 512, 64, 512
BL = B // N_CORES  # local batches per core
DC = D // 128  # d-chunks of 128 partitions
LC = L // 512  # l-chunks of 512 (matmul moving-operand max for 4-byte)
MASK_NEG = np.float16(-30000.0)


def _round_fp32r(a: np.ndarray) -> np.ndarray:
    """Round fp32 mantissa to 11 bits (RNE) — the PE's fp32r operand format."""
    u = np.ascontiguousarray(a, dtype=np.float32).view(np.uint32)
    lsb = (u >> np.uint32(12)) & np.uint32(1)
    u = (u + np.uint32(0x7FF) + lsb) & np.uint32(0xFFFFF000)
    return u.view(np.float32)


def _build_bass():
    nc = bacc.Bacc(
        "TRN2", target_bir_lowering=False, debug=False, num_devices=N_CORES
    )
    encT = nc.dram_tensor("encT", [BL * D, L], F32R, kind="ExternalInput")
    msk = nc.dram_tensor("msk", [1, BL * L], F8, kind="ExternalInput")
    hidT = nc.dram_tensor("hidT", [H, BL], F32, kind="ExternalInput")
    w_enc = nc.dram_tensor("w_enc", [D, A], F32R, kind="ExternalInput")
    w_hid = nc.dram_tensor("w_hid", [H, A], F32, kind="ExternalInput")
    b_attn = nc.dram_tensor("b_attn", [A, 1], F32, kind="ExternalInput")
    vrep = nc.dram_tensor("vrep", [A, 128], F16, kind="ExternalInput")
    ones1 = nc.dram_tensor("ones1", [1, 128], F8, kind="ExternalInput")
    ident = nc.dram_tensor("ident", [128, 128], F32, kind="ExternalInput")
    out = nc.dram_tensor("out", [BL, D], F32, kind="ExternalOutput")

    with tile.TileContext(nc) as tc, ExitStack() as ctx:
        const = ctx.enter_context(tc.tile_pool(name="const", bufs=1))
        encp = ctx.enter_context(tc.tile_pool(name="encp", bufs=4))
        tanhp = ctx.enter_context(tc.tile_pool(name="tanhp", bufs=3))
        pp = ctx.enter_context(tc.tile_pool(name="pp", bufs=2))
        scrp = ctx.enter_context(tc.tile_pool(name="scrp", bufs=2))
        smallp = ctx.enter_context(tc.tile_pool(name="smallp", bufs=4))
        resp = ctx.enter_context(tc.tile_pool(name="resp", bufs=1))
        ps_parts = ctx.enter_context(
            tc.tile_pool(name="ps_parts", bufs=2, space="PSUM")
        )
        ps_sc = ctx.enter_context(tc.tile_pool(name="ps_sc", bufs=2, space="PSUM"))
        ps_misc = ctx.enter_context(tc.tile_pool(name="ps_misc", bufs=1, space="PSUM"))

        # ---- one-time constants (Bacc's event-sem pass handles multi-waits) ----
        def loaded(shape, dtype, dram_ap):
            # consts go through the ACT-queue HWDGE so the sync queue can
            # start streaming enc immediately
            dst = const.tile(shape, dtype, tag="c_" + dram_ap.tensor.name)
            nc.scalar.dma_start(dst[:], dram_ap)
            return dst

        w_enc_sb = loaded(
            [128, DC, A], F32R, w_enc.ap().rearrange("(dc p) a -> p dc a", p=128)
        )
        w_hid_sb = loaded(
            [128, DC, A], F32, w_hid.ap().rearrange("(dc p) a -> p dc a", p=128)
        )
        hidT_sb = loaded(
            [128, DC, BL], F32, hidT.ap().rearrange("(dc p) b -> p dc b", p=128)
        )
        vrep_sb = loaded([A, 128], F16, vrep.ap())
        ones1_sb = loaded([1, 128], F8, ones1.ap())
        msk_sb = loaded([1, BL * L], F8, msk.ap())
        ident_sb = loaded([128, 128], F32, ident.ap())
        b_attn_sb = loaded([A, 1], F32, b_attn.ap())

        # hid = hidden @ W_hidden, transposed to [a, b] (tiny, full fp32)
        hid_ps = ps_misc.tile([A, BL], F32, tag="hid")
        for dc in range(DC):
            nc.tensor.matmul(
                hid_ps[:], lhsT=w_hid_sb[:, dc, :], rhs=hidT_sb[:, dc, :],
                start=(dc == 0), stop=(dc == DC - 1),
            )
        hplus_sb = const.tile([A, BL], F32)  # b_attn + hid.T, per-partition bias
        nc.vector.tensor_scalar_add(hplus_sb[:], hid_ps[:], b_attn_sb[:])

        res_all = resp.tile([128, BL * DC], F32)  # col j = b*DC + dc
        s_all = resp.tile([128, BL], F32)

        encT_v = encT.ap().rearrange("(b dc p) l -> b dc p l", dc=DC, p=128)

        for b in range(BL):
            et = encp.tile([128, DC, L], F32R, tag="et")
            for dc in range(DC):
                nc.sync.dma_start(et[:, dc, :], encT_v[b, dc, :, :])

            p_sb = pp.tile([128, L], F32, tag="p")
            s_half = [None, None]
            for h in range(2):
                sc_ps = ps_sc.tile([128, L // 2], F32, tag="sc")
                for half_lc in range(2):
                    lc = 2 * h + half_lc
                    lsl = slice(lc * 512, (lc + 1) * 512)
                    hsl = slice(half_lc * 512, (half_lc + 1) * 512)
                    parts_ps = ps_parts.tile([A, 512], F32, tag="parts")
                    for dc in range(DC):
                        nc.tensor.matmul(
                            parts_ps[:], lhsT=w_enc_sb[:, dc, :], rhs=et[:, dc, lsl],
                            start=(dc == 0), stop=(dc == DC - 1),
                        )
                    th = tanhp.tile([A, 512], F16, tag="th")
                    nc.scalar.activation(
                        th[:], parts_ps[:], mybir.ActivationFunctionType.Tanh,
                        bias=hplus_sb[:, b : b + 1],
                    )
                    # scores (broadcast to 128 partitions) + length mask
                    nc.tensor.matmul(
                        sc_ps[:, hsl], lhsT=vrep_sb[:], rhs=th[:],
                        start=True, stop=False,
                    )
                    nc.tensor.matmul(
                        sc_ps[:, hsl], lhsT=ones1_sb[:],
                        rhs=msk_sb[:, b * L + lc * 512 : b * L + (lc + 1) * 512],
                        start=False, stop=True,
                    )
                sh = smallp.tile([128, 1], F32, tag=f"sh{h}")
                nc.scalar.activation(
                    p_sb[:, h * 1024 : (h + 1) * 1024], sc_ps[:],
                    mybir.ActivationFunctionType.Exp, accum_out=sh[:],
                )
                s_half[h] = sh

            # softmax denominator: accumulate per-b, normalize after the loop
            nc.vector.tensor_add(s_all[:, b : b + 1], s_half[0][:], s_half[1][:])

            for dc in range(DC):
                scratch = scrp.tile([128, L], F32, tag="scratch")
                nc.vector.scalar_tensor_tensor(
                    out=scratch[:],
                    in0=et[:, dc, :].bitcast(F32),
                    scalar=1.0,
                    in1=p_sb[:],
                    op0=mybir.AluOpType.mult,
                    op1=mybir.AluOpType.mult,
                    accum_out=res_all[:, b * DC + dc : b * DC + dc + 1],
                )

        # normalize: res[:, b*DC:(b+1)*DC] /= S_b, then transpose + write out
        r_all = resp.tile([128, BL], F32)
        nc.vector.reciprocal(r_all[:], s_all[:])
        for b in range(BL):
            nc.vector.tensor_scalar_mul(
                res_all[:, b * DC : (b + 1) * DC],
                res_all[:, b * DC : (b + 1) * DC],
                r_all[:, b : b + 1],
            )
        t_ps = ps_misc.tile([BL * DC, 128], F32, tag="tps")
        nc.tensor.transpose(t_ps[:], res_all[:], ident_sb[:])
        out_sb = resp.tile([BL * DC, 128], F32)
        nc.vector.tensor_copy(out_sb[:], t_ps[:])
        nc.sync.dma_start(out.ap().rearrange("b (dc x) -> (b dc) x", x=128), out_sb[:])

    nc.compile()
    return nc


_NC_CACHE = None


def _get_nc():
    global _NC_CACHE
    if _NC_CACHE is None:
        _NC_CACHE = _build_bass()
    return _NC_CACHE


def prepare_in_maps(enc_outputs, lens, hidden_states, W_enc, b_attn, W_hidden, v):
    """Host-side sharding + layout transforms. Returns list of per-core maps."""
    enc_outputs = np.asarray(enc_outputs, dtype=np.float32)
    lens = np.asarray(lens, dtype=np.int32)
    hidden_states = np.asarray(hidden_states, dtype=np.float32)
    W_enc = np.asarray(W_enc, dtype=np.float32)
    b_attn = np.asarray(b_attn, dtype=np.float32)
    W_hidden = np.asarray(W_hidden, dtype=np.float32)
    v = np.asarray(v, dtype=np.float32)

    # (L, B, D) -> (B, D, L), contiguous, pre-rounded to fp32r
    encT = _round_fp32r(np.ascontiguousarray(enc_outputs.transpose(1, 2, 0)))
    w_enc_r = _round_fp32r(W_enc)
    import ml_dtypes
    vrep = np.ascontiguousarray(np.repeat(v.astype(np.float16)[:, None], 128, axis=1))
    ones1 = np.ones((1, 128), dtype=ml_dtypes.float8_e5m2)
    ident = np.eye(128, dtype=np.float32)
    b_attn_c = np.ascontiguousarray(b_attn[:, None])

    # length mask rows: 0 where l < lens[b], -30000 where l >= lens[b]
    li = np.arange(L, dtype=np.int32)[None, :]
    mask_full = np.where(li < lens[:, None], 0.0, -30000.0).astype(
        ml_dtypes.float8_e5m2
    )  # (B, L)

    hiddenT = np.ascontiguousarray(hidden_states.T)  # (H, B)

    in_maps = []
    for c in range(N_CORES):
        bs = slice(c * BL, (c + 1) * BL)
        in_maps.append(
            {
                "encT": encT[bs].reshape(BL * D, L),
                "msk": np.ascontiguousarray(mask_full[bs].reshape(1, BL * L)),
                "hidT": np.ascontiguousarray(hiddenT[:, bs]),
                "w_enc": w_enc_r,
                "w_hid": W_hidden,
                "b_attn": b_attn_c,
                "vrep": vrep,
                "ones1": ones1,
                "ident": ident,
            }
        )
    return in_maps


def kernel(enc_outputs, lens, hidden_states, W_enc, b_attn, W_hidden, v, **kwargs):
    nc = _get_nc()
    in_maps = prepare_in_maps(
        enc_outputs, lens, hidden_states, W_enc, b_attn, W_hidden, v
    )
    res = run_bass_kernel_spmd(nc, in_maps, core_ids=list(range(N_CORES)))
    return np.concatenate([res.results[c]["out"] for c in range(N_CORES)], axis=0)


def kernel_traced(enc_outputs, lens, hidden_states, W_enc, b_attn, W_hidden, v):
    """Like kernel() but returns (output, BassKernelResults with trace)."""
    nc = _get_nc()
    in_maps = prepare_in_maps(
        enc_outputs, lens, hidden_states, W_enc, b_attn, W_hidden, v
    )
    res = run_bass_kernel_spmd(nc, in_maps, core_ids=list(range(N_CORES)), trace=True)
    out = np.concatenate([res.results[c]["out"] for c in range(N_CORES)], axis=0)
    return out, res


# revision 14
# speedup vs baseline: 1.3350x; 1.2910x over previous
"""Bahdanau additive attention (ragged sequence) on 8 Trainium2 NeuronCores.

Reference math (per batch b over sequence l, d=512, a=64):
    parts  = enc @ W_enc + b_attn                        (l, a)
    scores = tanh(parts + hidden @ W_hidden) . v         (l,)
    w      = softmax(scores + mask) over l               (valid: l < lens[b])
    out[b] = sum_l w[l] * enc[l, b, :]                   (512,)

Strategy (batch-parallel over 8 cores, 8 batches each; single pass over enc):
  * Host pre-transposes enc to (b, d, l) and pre-rounds it to fp32r (fp32
    with 11-bit mantissa, RNE — the PE's full-rate 4-byte mode), so stage A
    (the d-contraction) streams natural [128, 512] tiles through the PE at
    1 col/cycle with W_enc chunks stationary.
  * Ragged skipping: sequence positions l >= lens[b] contribute exactly 0,
    so whole 512-wide chunks past ceil(lens/512) are never loaded or
    computed. The host sorts batches by chunk count and deals them across
    cores round-robin (balancing total work), and the kernel is compiled
    against the per-slot chunk-count template (max across cores per slot) —
    batches with fewer chunks than their slot just process extra masked
    chunks, which is still exact.
  * tanh runs on ACT (bias = b_attn + hidden@W_hidden per-partition), fp16
    out; the v-dot runs on PE with a column-replicated v [64, 128] so scores
    land broadcast across all 128 PSUM partitions. A second matmul
    accumulates a host-built 0/-28672 fp8 length mask into the same PSUM.
  * exp runs on ACT straight out of PSUM with accum_out giving the softmax
    denominator for free; max-subtraction is unnecessary because
    |scores| <= sum|v| < 60 (exp stays finite in fp32) and the +1.0 valid
    shift in the reference cancels in softmax.
  * Stage B (the l-contraction) runs on DVE as one scalar_tensor_tensor per
    (slot, d-chunk): accum_out[d] = sum_l enc[d, l] * P[l]; softmax
    normalization is deferred to a per-column scale at the very end so the
    per-batch critical path has no reciprocal in it.
  * The per-core result [128, 4*8] is PE-transposed once and DMA'd out; the
    host undoes the batch permutation.

Sync-wait note: matmuls with inline weight load allow only one HW wait;
Bacc's generate_event_semaphores pass splits excess waits automatically.
Constants are DMA'd via the ACT-queue HWDGE so the sync queue starts
streaming enc immediately.
"""
import sys

sys.path.insert(0, "/opt/trn_rl_repo")

from contextlib import ExitStack

import ml_dtypes
import numpy as np

import concourse.bacc as bacc
import concourse.bass as bass  # noqa: F401  (kept for debugging)
import concourse.tile as tile
from concourse import mybir
from concourse.bass_utils import run_bass_kernel_spmd

F32 = mybir.dt.float32
F32R = mybir.dt.float32r
F16 = mybir.dt.float16
F8 = mybir.dt.float8e5

N_CORES = 8
L, B, D, A, H = 2048, 64, 512, 64, 512
BL = B // N_CORES  # local batches per core
DC = D // 128  # d-chunks of 128 partitions
CHUNK = 512  # l-chunk width (matmul moving-operand max for 4-byte dtypes)
NCH = L // CHUNK


def _round_fp32r(a: np.ndarray) -> np.ndarray:
    """Round fp32 mantissa to 11 bits (RNE) — the PE's fp32r operand format."""
    u = np.ascontiguousarray(a, dtype=np.float32).view(np.uint32)
    lsb = (u >> np.uint32(12)) & np.uint32(1)
    u = (u + np.uint32(0x7FF) + lsb) & np.uint32(0xFFFFF000)
    return u.view(np.float32)


def _build_bass(template):
    """template: per-slot chunk counts (length BL, each 1..NCH)."""
    nc = bacc.Bacc(
        "TRN2", target_bir_lowering=False, debug=False, num_devices=N_CORES
    )
    encT = nc.dram_tensor("encT", [BL * D, L], F32R, kind="ExternalInput")
    msk = nc.dram_tensor("msk", [1, BL * L], F8, kind="ExternalInput")
    hidT = nc.dram_tensor("hidT", [H, BL], F32, kind="ExternalInput")
    w_enc = nc.dram_tensor("w_enc", [D, A], F32R, kind="ExternalInput")
    w_hid = nc.dram_tensor("w_hid", [H, A], F32, kind="ExternalInput")
    b_attn = nc.dram_tensor("b_attn", [A, 1], F32, kind="ExternalInput")
    vrep = nc.dram_tensor("vrep", [A, 128], F16, kind="ExternalInput")
    ones1 = nc.dram_tensor("ones1", [1, 128], F8, kind="ExternalInput")
    ident = nc.dram_tensor("ident", [128, 128], F32, kind="ExternalInput")
    out = nc.dram_tensor("out", [BL, D], F32, kind="ExternalOutput")

    with tile.TileContext(nc) as tc, ExitStack() as ctx:
        const = ctx.enter_context(tc.tile_pool(name="const", bufs=1))
        encp = ctx.enter_context(tc.tile_pool(name="encp", bufs=4))
        tanhp = ctx.enter_context(tc.tile_pool(name="tanhp", bufs=3))
        pp = ctx.enter_context(tc.tile_pool(name="pp", bufs=2))
        scrp = ctx.enter_context(tc.tile_pool(name="scrp", bufs=2))
        smallp = ctx.enter_context(tc.tile_pool(name="smallp", bufs=4))
        resp = ctx.enter_context(tc.tile_pool(name="resp", bufs=1))
        ps_parts = ctx.enter_context(
            tc.tile_pool(name="ps_parts", bufs=2, space="PSUM")
        )
        ps_sc = ctx.enter_context(tc.tile_pool(name="ps_sc", bufs=2, space="PSUM"))
        ps_misc = ctx.enter_context(tc.tile_pool(name="ps_misc", bufs=1, space="PSUM"))

        # ---- one-time constants on the ACT-queue HWDGE ----
        def loaded(shape, dtype, dram_ap):
            dst = const.tile(shape, dtype, tag="c_" + dram_ap.tensor.name)
            nc.scalar.dma_start(dst[:], dram_ap)
            return dst

        w_enc_sb = loaded(
            [128, DC, A], F32R, w_enc.ap().rearrange("(dc p) a -> p dc a", p=128)
        )
        w_hid_sb = loaded(
            [128, DC, A], F32, w_hid.ap().rearrange("(dc p) a -> p dc a", p=128)
        )
        hidT_sb = loaded(
            [128, DC, BL], F32, hidT.ap().rearrange("(dc p) b -> p dc b", p=128)
        )
        vrep_sb = loaded([A, 128], F16, vrep.ap())
        ones1_sb = loaded([1, 128], F8, ones1.ap())
        msk_sb = loaded([1, BL * L], F8, msk.ap())
        ident_sb = loaded([128, 128], F32, ident.ap())
        b_attn_sb = loaded([A, 1], F32, b_attn.ap())

        # hid = hidden @ W_hidden, transposed to [a, b] (tiny, full fp32)
        hid_ps = ps_misc.tile([A, BL], F32, tag="hid")
        for dc in range(DC):
            nc.tensor.matmul(
                hid_ps[:], lhsT=w_hid_sb[:, dc, :], rhs=hidT_sb[:, dc, :],
                start=(dc == 0), stop=(dc == DC - 1),
            )
        hplus_sb = const.tile([A, BL], F32)  # b_attn + hid.T, per-partition bias
        nc.vector.tensor_scalar_add(hplus_sb[:], hid_ps[:], b_attn_sb[:])

        res_all = resp.tile([128, BL * DC], F32)  # col j = slot*DC + dc
        s_all = resp.tile([128, BL], F32)

        encT_v = encT.ap().rearrange("(b dc p) l -> b dc p l", dc=DC, p=128)

        for j in range(BL):
            C = int(template[j])
            n_l = C * CHUNK
            et = encp.tile([128, DC, L], F32R, tag="et")
            for dc in range(DC):
                nc.sync.dma_start(et[:, dc, 0:n_l], encT_v[j, dc, :, 0:n_l])

            p_sb = pp.tile([128, L], F32, tag="p")
            halves = [(0, min(C, 2))]
            if C > 2:
                halves.append((2, C))
            s_half = []
            for h, (c0, c1) in enumerate(halves):
                nh = (c1 - c0) * CHUNK
                sc_ps = ps_sc.tile([128, 2 * CHUNK], F32, tag="sc")
                for lc in range(c0, c1):
                    lsl = slice(lc * CHUNK, (lc + 1) * CHUNK)
                    hsl = slice((lc - c0) * CHUNK, (lc - c0 + 1) * CHUNK)
                    parts_ps = ps_parts.tile([A, CHUNK], F32, tag="parts")
                    for dc in range(DC):
                        nc.tensor.matmul(
                            parts_ps[:], lhsT=w_enc_sb[:, dc, :], rhs=et[:, dc, lsl],
                            start=(dc == 0), stop=(dc == DC - 1),
                        )
                    th = tanhp.tile([A, CHUNK], F16, tag="th")
                    nc.scalar.activation(
                        th[:], parts_ps[:], mybir.ActivationFunctionType.Tanh,
                        bias=hplus_sb[:, j : j + 1],
                    )
                    # scores (broadcast to 128 partitions) + length mask
                    nc.tensor.matmul(
                        sc_ps[:, hsl], lhsT=vrep_sb[:], rhs=th[:],
                        start=True, stop=False,
                    )
                    nc.tensor.matmul(
                        sc_ps[:, hsl], lhsT=ones1_sb[:],
                        rhs=msk_sb[:, j * L + lc * CHUNK : j * L + (lc + 1) * CHUNK],
                        start=False, stop=True,
                    )
                sh = smallp.tile([128, 1], F32, tag=f"sh{h}")
                nc.scalar.activation(
                    p_sb[:, c0 * CHUNK : c0 * CHUNK + nh], sc_ps[:, 0:nh],
                    mybir.ActivationFunctionType.Exp, accum_out=sh[:],
                )
                s_half.append(sh)

            # softmax denominator: accumulate per-slot, normalize after the loop
            if len(s_half) == 2:
                nc.vector.tensor_add(s_all[:, j : j + 1], s_half[0][:], s_half[1][:])
            else:
                nc.vector.tensor_copy(s_all[:, j : j + 1], s_half[0][:])

            for dc in range(DC):
                scratch = scrp.tile([128, L], F32, tag="scratch")
                nc.vector.scalar_tensor_tensor(
                    out=scratch[:, 0:n_l],
                    in0=et[:, dc, 0:n_l].bitcast(F32),
                    scalar=1.0,
                    in1=p_sb[:, 0:n_l],
                    op0=mybir.AluOpType.mult,
                    op1=mybir.AluOpType.mult,
                    accum_out=res_all[:, j * DC + dc : j * DC + dc + 1],
                )

        # normalize: res[:, j*DC:(j+1)*DC] *= 1/S_j, then transpose + write out
        r_all = resp.tile([128, BL], F32)
        nc.vector.reciprocal(r_all[:], s_all[:])
        for j in range(BL):
            nc.vector.tensor_scalar_mul(
                res_all[:, j * DC : (j + 1) * DC],
                res_all[:, j * DC : (j + 1) * DC],
                r_all[:, j : j + 1],
            )
        t_ps = ps_misc.tile([BL * DC, 128], F32, tag="tps")
        nc.tensor.transpose(t_ps[:], res_all[:], ident_sb[:])
        out_sb = resp.tile([BL * DC, 128], F32)
        nc.vector.tensor_copy(out_sb[:], t_ps[:])
        nc.sync.dma_start(out.ap().rearrange("b (dc x) -> (b dc) x", x=128), out_sb[:])

    nc.compile()
    return nc


_NC_CACHE = {}


def _get_nc(template):
    key = tuple(int(c) for c in template)
    if key not in _NC_CACHE:
        _NC_CACHE[key] = _build_bass(key)
    return _NC_CACHE[key]


def _plan(lens):
    """Balance batches across cores by valid-chunk count.

    Returns (assign, template): assign[c][j] = original batch index handled
    by core c, slot j; template[j] = chunks compiled for slot j (max over
    cores of that slot's batch need).
    """
    chunks = np.minimum(np.ceil(np.maximum(lens, 1) / CHUNK).astype(int), NCH)
    order = np.argsort(-chunks, kind="stable")  # descending need
    assign = [[int(order[j * N_CORES + c]) for j in range(BL)] for c in range(N_CORES)]
    template = tuple(
        int(chunks[order[j * N_CORES]]) for j in range(BL)
    )  # first in each row is the row max (sorted order)
    return assign, template


def prepare_in_maps(enc_outputs, lens, hidden_states, W_enc, b_attn, W_hidden, v):
    """Host-side sharding + layout transforms. Returns (in_maps, assign)."""
    enc_outputs = np.asarray(enc_outputs, dtype=np.float32)
    lens = np.asarray(lens, dtype=np.int32)
    hidden_states = np.asarray(hidden_states, dtype=np.float32)
    W_enc = np.asarray(W_enc, dtype=np.float32)
    b_attn = np.asarray(b_attn, dtype=np.float32)
    W_hidden = np.asarray(W_hidden, dtype=np.float32)
    v = np.asarray(v, dtype=np.float32)

    assign, template = _plan(lens)

    # (L, B, D) -> (B, D, L), contiguous, pre-rounded to fp32r
    encT = _round_fp32r(np.ascontiguousarray(enc_outputs.transpose(1, 2, 0)))
    w_enc_r = _round_fp32r(W_enc)
    vrep = np.ascontiguousarray(np.repeat(v.astype(np.float16)[:, None], 128, axis=1))
    ones1 = np.ones((1, 128), dtype=ml_dtypes.float8_e5m2)
    ident = np.eye(128, dtype=np.float32)
    b_attn_c = np.ascontiguousarray(b_attn[:, None])

    # length mask rows: 0 where l < lens[b], -28672 where l >= lens[b]
    li = np.arange(L, dtype=np.int32)[None, :]
    mask_full = np.where(li < lens[:, None], 0.0, -30000.0).astype(
        ml_dtypes.float8_e5m2
    )  # (B, L)

    hiddenT = hidden_states.T  # (H, B)

    in_maps = []
    for c in range(N_CORES):
        bs = assign[c]
        in_maps.append(
            {
                "encT": np.ascontiguousarray(encT[bs]).reshape(BL * D, L),
                "msk": np.ascontiguousarray(mask_full[bs]).reshape(1, BL * L),
                "hidT": np.ascontiguousarray(hiddenT[:, bs]),
                "w_enc": w_enc_r,
                "w_hid": W_hidden,
                "b_attn": b_attn_c,
                "vrep": vrep,
                "ones1": ones1,
                "ident": ident,
            }
        )
    return in_maps, assign, template


def _run(inputs_np, trace=False):
    in_maps, assign, template = prepare_in_maps(**inputs_np)
    nc = _get_nc(template)
    res = run_bass_kernel_spmd(
        nc, in_maps, core_ids=list(range(N_CORES)), trace=trace
    )
    out = np.empty((B, D), dtype=np.float32)
    for c in range(N_CORES):
        rows = res.results[c]["out"]
        for j in range(BL):
            out[assign[c][j]] = rows[j]
    return out, res


def kernel(enc_outputs, lens, hidden_states, W_enc, b_attn, W_hidden, v, **kwargs):
    out, _ = _run(
        dict(
            enc_outputs=enc_outputs, lens=lens, hidden_states=hidden_states,
            W_enc=W_enc, b_attn=b_attn, W_hidden=W_hidden, v=v,
        )
    )
    return out


def kernel_traced(enc_outputs, lens, hidden_states, W_enc, b_attn, W_hidden, v):
    """Like kernel() but returns (output, BassKernelResults with trace)."""
    return _run(
        dict(
            enc_outputs=enc_outputs, lens=lens, hidden_states=hidden_states,
            W_enc=W_enc, b_attn=b_attn, W_hidden=W_hidden, v=v,
        ),
        trace=True,
    )


# revision 15
# speedup vs baseline: 1.5102x; 1.1312x over previous
"""Bahdanau additive attention (ragged sequence) on 8 Trainium2 NeuronCores.

Reference math (per batch b over sequence l, d=512, a=64):
    parts  = enc @ W_enc + b_attn                        (l, a)
    scores = tanh(parts + hidden @ W_hidden) . v         (l,)
    w      = softmax(scores + mask) over l               (valid: l < lens[b])
    out[b] = sum_l w[l] * enc[l, b, :]                   (512,)

Strategy (batch-parallel over 8 cores, 8 batches each; single pass over enc):
  * Host pre-transposes enc to (b, d, l) and pre-rounds it to fp32r (fp32
    with 11-bit mantissa, RNE — the PE's full-rate 4-byte mode), so stage A
    (the d-contraction) streams natural [128, 512] tiles through the PE at
    1 col/cycle with W_enc chunks stationary.
  * Ragged skipping: sequence positions l >= lens[b] contribute exactly 0,
    so whole 512-wide chunks past ceil(lens/512) are never loaded or
    computed. The host sorts batches by chunk count and deals them across
    cores round-robin (balancing total work), and the kernel is compiled
    against the per-slot chunk-count template (max across cores per slot) —
    batches with fewer chunks than their slot just process extra masked
    chunks, which is still exact.
  * tanh runs on ACT (bias = b_attn + hidden@W_hidden per-partition), fp16
    out; the v-dot runs on PE with a column-replicated v [64, 128] so scores
    land broadcast across all 128 PSUM partitions. A second matmul
    accumulates a host-built 0/-28672 fp8 length mask into the same PSUM.
  * exp runs on ACT straight out of PSUM with accum_out giving the softmax
    denominator for free; max-subtraction is unnecessary because
    |scores| <= sum|v| < 60 (exp stays finite in fp32) and the +1.0 valid
    shift in the reference cancels in softmax.
  * Stage B (the l-contraction) runs on DVE as one scalar_tensor_tensor per
    (slot, d-chunk): accum_out[d] = sum_l enc[d, l] * P[l]; softmax
    normalization is deferred to a per-column scale at the very end so the
    per-batch critical path has no reciprocal in it.
  * The per-core result [128, 4*8] is PE-transposed once and DMA'd out; the
    host undoes the batch permutation.

Sync-wait note: matmuls with inline weight load allow only one HW wait;
Bacc's generate_event_semaphores pass splits excess waits automatically.
Constants are DMA'd via the ACT-queue HWDGE so the sync queue starts
streaming enc immediately.
"""
import sys

sys.path.insert(0, "/opt/trn_rl_repo")

from contextlib import ExitStack

import ml_dtypes
import numpy as np

import concourse.bacc as bacc
import concourse.bass as bass  # noqa: F401  (kept for debugging)
import concourse.tile as tile
from concourse import mybir
from concourse.bass_utils import run_bass_kernel_spmd

F32 = mybir.dt.float32
F32R = mybir.dt.float32r
F16 = mybir.dt.float16
F8 = mybir.dt.float8e5

N_CORES = 8
L, B, D, A, H = 2048, 64, 512, 64, 512
BL = B // N_CORES  # local batches per core
DC = D // 128  # d-chunks of 128 partitions
CHUNK = 512  # l-chunk width (matmul moving-operand max for 4-byte dtypes)
NCH = L // CHUNK


def _round_fp32r(a: np.ndarray) -> np.ndarray:
    """Round fp32 mantissa to 11 bits (RNE) — the PE's fp32r operand format."""
    u = np.ascontiguousarray(a, dtype=np.float32).view(np.uint32)
    lsb = (u >> np.uint32(12)) & np.uint32(1)
    u = (u + np.uint32(0x7FF) + lsb) & np.uint32(0xFFFFF000)
    return u.view(np.float32)


def _build_bass(template):
    """template: per-slot chunk counts (length BL, each 1..NCH)."""
    nc = bacc.Bacc(
        "TRN2", target_bir_lowering=False, debug=False, num_devices=N_CORES
    )
    encT = nc.dram_tensor("encT", [BL * D, L], F16, kind="ExternalInput")
    msk = nc.dram_tensor("msk", [1, BL * L], F8, kind="ExternalInput")
    hidT = nc.dram_tensor("hidT", [H, BL], F32, kind="ExternalInput")
    w_enc = nc.dram_tensor("w_enc", [D, A], F16, kind="ExternalInput")
    w_hid = nc.dram_tensor("w_hid", [H, A], F32, kind="ExternalInput")
    b_attn = nc.dram_tensor("b_attn", [A, 1], F32, kind="ExternalInput")
    vrep = nc.dram_tensor("vrep", [A, 128], F16, kind="ExternalInput")
    ones1 = nc.dram_tensor("ones1", [1, 128], F8, kind="ExternalInput")
    ident = nc.dram_tensor("ident", [128, 128], F32, kind="ExternalInput")
    out = nc.dram_tensor("out", [BL, D], F32, kind="ExternalOutput")

    with tile.TileContext(nc) as tc, ExitStack() as ctx:
        const = ctx.enter_context(tc.tile_pool(name="const", bufs=1))
        encp = ctx.enter_context(tc.tile_pool(name="encp", bufs=4))
        tanhp = ctx.enter_context(tc.tile_pool(name="tanhp", bufs=3))
        pp = ctx.enter_context(tc.tile_pool(name="pp", bufs=2))
        scrp = ctx.enter_context(tc.tile_pool(name="scrp", bufs=2))
        smallp = ctx.enter_context(tc.tile_pool(name="smallp", bufs=4))
        resp = ctx.enter_context(tc.tile_pool(name="resp", bufs=1))
        ps_parts = ctx.enter_context(
            tc.tile_pool(name="ps_parts", bufs=2, space="PSUM")
        )
        ps_sc = ctx.enter_context(tc.tile_pool(name="ps_sc", bufs=2, space="PSUM"))
        ps_misc = ctx.enter_context(tc.tile_pool(name="ps_misc", bufs=1, space="PSUM"))

        # ---- one-time constants on the ACT-queue HWDGE ----
        def loaded(shape, dtype, dram_ap):
            dst = const.tile(shape, dtype, tag="c_" + dram_ap.tensor.name)
            nc.scalar.dma_start(dst[:], dram_ap)
            return dst

        w_enc_sb = loaded(
            [128, DC, A], F16, w_enc.ap().rearrange("(dc p) a -> p dc a", p=128)
        )
        w_hid_sb = loaded(
            [128, DC, A], F32, w_hid.ap().rearrange("(dc p) a -> p dc a", p=128)
        )
        hidT_sb = loaded(
            [128, DC, BL], F32, hidT.ap().rearrange("(dc p) b -> p dc b", p=128)
        )
        vrep_sb = loaded([A, 128], F16, vrep.ap())
        ones1_sb = loaded([1, 128], F8, ones1.ap())
        msk_sb = loaded([1, BL * L], F8, msk.ap())
        ident_sb = loaded([128, 128], F32, ident.ap())
        b_attn_sb = loaded([A, 1], F32, b_attn.ap())

        # hid = hidden @ W_hidden, transposed to [a, b] (tiny, full fp32)
        hid_ps = ps_misc.tile([A, BL], F32, tag="hid")
        for dc in range(DC):
            nc.tensor.matmul(
                hid_ps[:], lhsT=w_hid_sb[:, dc, :], rhs=hidT_sb[:, dc, :],
                start=(dc == 0), stop=(dc == DC - 1),
            )
        hplus_sb = const.tile([A, BL], F32)  # b_attn + hid.T, per-partition bias
        nc.vector.tensor_scalar_add(hplus_sb[:], hid_ps[:], b_attn_sb[:])

        res_all = resp.tile([128, BL * DC], F32)  # col j = slot*DC + dc
        s_all = resp.tile([128, BL], F32)

        encT_v = encT.ap().rearrange("(b dc p) l -> b dc p l", dc=DC, p=128)

        for j in range(BL):
            C = int(template[j])
            n_l = C * CHUNK
            et = encp.tile([128, DC, L], F16, tag="et")
            for dc in range(DC):
                nc.sync.dma_start(et[:, dc, 0:n_l], encT_v[j, dc, :, 0:n_l])

            p_sb = pp.tile([128, L], F32, tag="p")
            halves = [(0, min(C, 2))]
            if C > 2:
                halves.append((2, C))
            s_half = []
            for h, (c0, c1) in enumerate(halves):
                nh = (c1 - c0) * CHUNK
                sc_ps = ps_sc.tile([128, 2 * CHUNK], F32, tag="sc")
                for lc in range(c0, c1):
                    lsl = slice(lc * CHUNK, (lc + 1) * CHUNK)
                    hsl = slice((lc - c0) * CHUNK, (lc - c0 + 1) * CHUNK)
                    parts_ps = ps_parts.tile([A, CHUNK], F32, tag="parts")
                    for dc in range(DC):
                        nc.tensor.matmul(
                            parts_ps[:], lhsT=w_enc_sb[:, dc, :], rhs=et[:, dc, lsl],
                            start=(dc == 0), stop=(dc == DC - 1),
                        )
                    th = tanhp.tile([A, CHUNK], F16, tag="th")
                    nc.scalar.activation(
                        th[:], parts_ps[:], mybir.ActivationFunctionType.Tanh,
                        bias=hplus_sb[:, j : j + 1],
                    )
                    # scores (broadcast to 128 partitions) + length mask
                    nc.tensor.matmul(
                        sc_ps[:, hsl], lhsT=vrep_sb[:], rhs=th[:],
                        start=True, stop=False,
                    )
                    nc.tensor.matmul(
                        sc_ps[:, hsl], lhsT=ones1_sb[:],
                        rhs=msk_sb[:, j * L + lc * CHUNK : j * L + (lc + 1) * CHUNK],
                        start=False, stop=True,
                    )
                sh = smallp.tile([128, 1], F32, tag=f"sh{h}")
                nc.scalar.activation(
                    p_sb[:, c0 * CHUNK : c0 * CHUNK + nh], sc_ps[:, 0:nh],
                    mybir.ActivationFunctionType.Exp, accum_out=sh[:],
                )
                s_half.append(sh)

            # softmax denominator: accumulate per-slot, normalize after the loop
            if len(s_half) == 2:
                nc.vector.tensor_add(s_all[:, j : j + 1], s_half[0][:], s_half[1][:])
            else:
                nc.vector.tensor_copy(s_all[:, j : j + 1], s_half[0][:])

            for dc in range(DC):
                scratch = scrp.tile([128, L], F32, tag="scratch")
                nc.vector.scalar_tensor_tensor(
                    out=scratch[:, 0:n_l],
                    in0=et[:, dc, 0:n_l],
                    scalar=1.0,
                    in1=p_sb[:, 0:n_l],
                    op0=mybir.AluOpType.mult,
                    op1=mybir.AluOpType.mult,
                    accum_out=res_all[:, j * DC + dc : j * DC + dc + 1],
                )

        # normalize: res[:, j*DC:(j+1)*DC] *= 1/S_j, then transpose + write out
        r_all = resp.tile([128, BL], F32)
        nc.vector.reciprocal(r_all[:], s_all[:])
        for j in range(BL):
            nc.vector.tensor_scalar_mul(
                res_all[:, j * DC : (j + 1) * DC],
                res_all[:, j * DC : (j + 1) * DC],
                r_all[:, j : j + 1],
            )
        t_ps = ps_misc.tile([BL * DC, 128], F32, tag="tps")
        nc.tensor.transpose(t_ps[:], res_all[:], ident_sb[:])
        out_sb = resp.tile([BL * DC, 128], F32)
        nc.vector.tensor_copy(out_sb[:], t_ps[:])
        nc.sync.dma_start(out.ap().rearrange("b (dc x) -> (b dc) x", x=128), out_sb[:])

    nc.compile()
    return nc


_NC_CACHE = {}


def _get_nc(template):
    key = tuple(int(c) for c in template)
    if key not in _NC_CACHE:
        _NC_CACHE[key] = _build_bass(key)
    return _NC_CACHE[key]


def _plan(lens):
    """Balance batches across cores by valid-chunk count.

    Returns (assign, template): assign[c][j] = original batch index handled
    by core c, slot j; template[j] = chunks compiled for slot j (max over
    cores of that slot's batch need).
    """
    chunks = np.minimum(np.ceil(np.maximum(lens, 1) / CHUNK).astype(int), NCH)
    order = np.argsort(-chunks, kind="stable")  # descending need
    # rank r (0=biggest group) -> slot position: put the second-smallest
    # group first so the pipeline primes fast, keep the smallest last so the
    # drain tail is short, biggest groups in the middle.
    ranks = list(range(BL))
    slot_ranks = [ranks[-2]] + ranks[: BL - 2] + [ranks[-1]]
    assign = [
        [int(order[r * N_CORES + c]) for r in slot_ranks] for c in range(N_CORES)
    ]
    template = tuple(int(chunks[order[r * N_CORES]]) for r in slot_ranks)
    return assign, template


def prepare_in_maps(enc_outputs, lens, hidden_states, W_enc, b_attn, W_hidden, v):
    """Host-side sharding + layout transforms. Returns (in_maps, assign)."""
    enc_outputs = np.asarray(enc_outputs, dtype=np.float32)
    lens = np.asarray(lens, dtype=np.int32)
    hidden_states = np.asarray(hidden_states, dtype=np.float32)
    W_enc = np.asarray(W_enc, dtype=np.float32)
    b_attn = np.asarray(b_attn, dtype=np.float32)
    W_hidden = np.asarray(W_hidden, dtype=np.float32)
    v = np.asarray(v, dtype=np.float32)

    assign, template = _plan(lens)

    # (L, B, D) -> (B, D, L), contiguous, fp16 (halves the HBM traffic; the
    # softmax weights and stage-B accumulation stay fp32)
    encT = np.ascontiguousarray(enc_outputs.transpose(1, 2, 0).astype(np.float16))
    w_enc_r = W_enc.astype(np.float16)
    vrep = np.ascontiguousarray(np.repeat(v.astype(np.float16)[:, None], 128, axis=1))
    ones1 = np.ones((1, 128), dtype=ml_dtypes.float8_e5m2)
    ident = np.eye(128, dtype=np.float32)
    b_attn_c = np.ascontiguousarray(b_attn[:, None])

    # length mask rows: 0 where l < lens[b], -28672 where l >= lens[b]
    li = np.arange(L, dtype=np.int32)[None, :]
    mask_full = np.where(li < lens[:, None], 0.0, -30000.0).astype(
        ml_dtypes.float8_e5m2
    )  # (B, L)

    hiddenT = hidden_states.T  # (H, B)

    in_maps = []
    for c in range(N_CORES):
        bs = assign[c]
        in_maps.append(
            {
                "encT": np.ascontiguousarray(encT[bs]).reshape(BL * D, L),
                "msk": np.ascontiguousarray(mask_full[bs]).reshape(1, BL * L),
                "hidT": np.ascontiguousarray(hiddenT[:, bs]),
                "w_enc": w_enc_r,
                "w_hid": W_hidden,
                "b_attn": b_attn_c,
                "vrep": vrep,
                "ones1": ones1,
                "ident": ident,
            }
        )
    return in_maps, assign, template


def _run(inputs_np, trace=False):
    in_maps, assign, template = prepare_in_maps(**inputs_np)
    nc = _get_nc(template)
    res = run_bass_kernel_spmd(
        nc, in_maps, core_ids=list(range(N_CORES)), trace=trace
    )
    out = np.empty((B, D), dtype=np.float32)
    for c in range(N_CORES):
        rows = res.results[c]["out"]
        for j in range(BL):
            out[assign[c][j]] = rows[j]
    return out, res


def kernel(enc_outputs, lens, hidden_states, W_enc, b_attn, W_hidden, v, **kwargs):
    out, _ = _run(
        dict(
            enc_outputs=enc_outputs, lens=lens, hidden_states=hidden_states,
            W_enc=W_enc, b_attn=b_attn, W_hidden=W_hidden, v=v,
        )
    )
    return out


def kernel_traced(enc_outputs, lens, hidden_states, W_enc, b_attn, W_hidden, v):
    """Like kernel() but returns (output, BassKernelResults with trace)."""
    return _run(
        dict(
            enc_outputs=enc_outputs, lens=lens, hidden_states=hidden_states,
            W_enc=W_enc, b_attn=b_attn, W_hidden=W_hidden, v=v,
        ),
        trace=True,
    )
